# revision 33
# baseline (speedup 1.0000x reference)
"""Causal grouped Conv1d on 8 Trainium2 NeuronCores.

Problem: x [B=4, L=4096, D=2048] f32, w [K=4, D/G=256, D=2048] f32, G=8 groups.
out[b, l, o] = sum_{k, i} x[b, l-3+k, g(o)*256 + i] * w[k, i, o]   (causal pad 3)

Sharding: hybrid tensor/data parallel — core c = (th, gp) with th = c // 4,
gp = c % 4 handles batches {2*th, 2*th+1} x channel slice [gp*512, gp*512+512)
(= groups 2*gp, 2*gp+1).

The host pre-transposes each core's x slice to channel-major bf16
[128 part, 4 chunks, 2 batches x (4 zero pad cols + 4096 token cols)], so the
device kernel is a pure conv-matmul stream — no PE transposes at all:
  psum[128 och, 512 tok] += w[k, cin, och].T @ xT[cin, k-window]
accumulated over k=0..3 and 2 cin chunks (contraction 1024 = 8 passes).
512 matmuls/core x 512 moving rows @ 1 cyc/row (bf16) = 262k PE cycles
(~109 us @ 2.4 GHz) — the compute roofline for this problem.

Schedule: a HAM warm-up (dummy matmuls) keeps the PE clock at 2.4 GHz through
the startup DMA wait; startup-critical tiles ride the sync hardware-DGE ring
in exact consumption order (w packed j-major per och quarter so dependency
intervals stay exact); psum drains alternate scalar/vector with y DMAs on
the scalar/sync hardware rings. Output is written och-major [512, 8192] bf16
per core; host upcasts and transposes back.
"""

import numpy as np
import ml_dtypes

import concourse.mybir as mybir
import concourse.tile as tile
from concourse import bacc
from concourse.bass_utils import run_bass_kernel_spmd

B, L, D, K, G = 4, 4096, 2048, 4, 8
CG = D // G               # 256 channels per group (in and out)
NCORES = 8
BPC = 2                   # batches per core
CPC = 512                 # channels per core (2 groups)
TOKC = BPC * L            # 8192 tokens per core
NCHUNK = CPC // 128       # 4 cin chunks of 128 per core
PAD = 4                   # leading zero cols per batch (>= K-1, 8B aligned)
CB = PAD + L              # 4100 cols per batch
XC = BPC * CB             # 8200 cols per chunk

TB = 512                  # token block (matmul moving dim / 1 psum bank f32)
NB_PER_B = L // TB        # 8 blocks per batch
NB = BPC * NB_PER_B       # 16 blocks per core
WQ = 2 * K * 128          # w cols per och quarter (och-quarter-major packing)

BF16 = mybir.dt.bfloat16
F32 = mybir.dt.float32
NPBF16 = np.dtype(ml_dtypes.bfloat16)


def _emit(tc, nc, xh, xgh, wh, y):
    """xh [128, NCHUNK, XC] bf16; wh [128, NCHUNK*WQ] bf16; y [CPC, TOKC] bf16."""
    import contextlib
    ctx = contextlib.ExitStack()
    with ctx:
        xp = ctx.enter_context(tc.tile_pool(name="xp", bufs=1))
        wp = ctx.enter_context(tc.tile_pool(name="wp", bufs=1))
        outp = ctx.enter_context(tc.tile_pool(name="outp", bufs=20))
        po = ctx.enter_context(tc.tile_pool(name="po", bufs=8, space="PSUM"))

        xbig = xp.tile([128, NCHUNK * XC], BF16, name="xbig")
        wall = wp.tile([128, 2 * K * CPC], BF16, name="wall")
        scr = wp.tile([128, TB], BF16, name="scr")
        # Dedicated gate tile for block 0: all 4 chunks' first-block cols
        # packed contiguously, so the gate DMA is 128 fat (4 KB) descriptors
        # instead of 512 thin ones — the early DMA ring is latency-bound
        # per descriptor. Block 1's halo reads still come from xbig via the
        # (later, uncritical) seg-0 DMA.
        GC = PAD + TB  # 516 gate cols per chunk
        xg = wp.tile([128, NCHUNK * GC], BF16, name="xg")

        # Startup-critical DMAs first, split so the first conv group's gate
        # is only ~0.5 MB: sync ring carries w's cc=0 och-quarter then x
        # block 0 chunks 0-1 (all the first group needs); scalar ring
        # carries the rest of w in parallel.
        xv = xbig[:].rearrange("p (ci c) -> p ci c", c=XC)
        # w is packed j-major within och quarters (col = cc*1024 + j*512 +
        # k*128 + oc) so every early dependency is one contiguous col range.
        # Both rings are loaded in first-consumption order: the first conv
        # group's j=0 half needs only 0.25 MB before matmuls can start.
        # All startup-critical pieces ride the sync ring (its start time is
        # reliable; the scalar ring's varies by multiple us) in exact
        # consumption order. Only w for cc2/3 — not needed until ~7us into
        # the stream — rides the scalar ring.
        HB = K * 128  # 512 cols per (cc, j) half
        nc.sync.dma_start(wall[:, 0:WQ], wh[:, 0:WQ])                  # cc0
        nc.sync.dma_start(xg[:], xgh[:])                               # block 0
        nc.sync.dma_start(wall[:, WQ:2 * WQ], wh[:, WQ:2 * WQ])        # cc1
        nc.sync.dma_start(xv[:, :, 0:PAD + TB], xh[:, :, 0:PAD + TB])  # halo
        nc.scalar.dma_start(wall[:, 2 * WQ:4 * WQ], wh[:, 2 * WQ:4 * WQ])

        # Remaining x segments (sync queue) in consumption order; each
        # covers all 4 chunks of one token block (+ batch-1's pad cols).
        for t in range(1, NB):
            b, tb = divmod(t, NB_PER_B)
            c0 = b * CB + (0 if tb == 0 else PAD + tb * TB)
            c1 = b * CB + PAD + (tb + 1) * TB
            nc.sync.dma_start(xv[:, :, c0:c1], xh[:, :, c0:c1])

        # HAM warm-up: dummy matmuls on a zeroed scratch tile keep the PE
        # busy through the startup DMA wait so the clock gate is at 8/8
        # (2.4 GHz) when the real stream begins (it would otherwise spend
        # its first ~3.4us at 1.2 GHz).
        nc.gpsimd.memset(scr[:], 0)
        wupot = po.tile([128, TB], F32, name="pot")  # shares the pot rotation
        for _ in range(12):
            nc.tensor.matmul(wupot[:], scr[:, 0:128], scr[:], start=True,
                             stop=True)

        # Conv matmul stream, block-sequential (LDWEIGHTS hides fully behind
        # the 512-cycle moving streams, so no stationary reuse is needed);
        # 8 psum banks hold 8 in-flight (block, och-chunk) groups while
        # scalar/vector drain them to SBUF.
        ncopy = 0
        for t in range(NB):
            b, tb = divmod(t, NB_PER_B)
            for cc in range(NCHUNK):
                gg = cc // 2  # local group of this och chunk
                last = t == NB - 1 and cc == NCHUNK - 1
                # The final group runs as two half-token psum groups so its
                # drain parallelizes across engines (readers of one psum
                # tile are serialized); same PE rows either way.
                H = TB // 2 if last else TB
                pots = [po.tile([128, H], F32, name="pot")
                        for _ in range(TB // H)]
                # Half-groups run sequentially (not interleaved) so the
                # first half's drain overlaps the second half's matmuls.
                for i, pot in enumerate(pots):
                    for j in range(2):
                        ch = 2 * gg + j
                        c0 = ch * XC + b * CB + 1 + tb * TB
                        for k in range(K):
                            wc0 = cc * WQ + (j * K + k) * 128
                            if t == 0:
                                rhs = xg[:, ch * GC + 1 + k:
                                         ch * GC + 1 + k + H]
                            else:
                                rhs = xbig[:, c0 + i * H + k:
                                           c0 + i * H + k + H]
                            nc.tensor.matmul(
                                pot[:], wall[:, wc0:wc0 + 128], rhs,
                                start=(j == 0 and k == 0),
                                stop=(j == 1 and k == K - 1),
                            )
                ydst = y[cc * 128:(cc + 1) * 128, t * TB:(t + 1) * TB]
                # psum->sbuf drain alternates scalar/vector; the y DMA
                # rides the hardware DGE rings (scalar issues its own
                # copies' DMAs in queue order; sync issues vector's,
                # queued behind the 16 upfront x issues).
                if last:
                    ot = outp.tile([128, TB], BF16, name="ot")
                    ot2 = outp.tile([128, TB], BF16, name="ot")
                    nc.scalar.copy(ot[:, 0:H], pots[0][:])
                    nc.vector.tensor_copy(ot2[:, 0:H], pots[1][:])
                    nc.scalar.dma_start(ydst[:, 0:H], ot[:, 0:H])
                    nc.sync.dma_start(ydst[:, H:], ot2[:, 0:H])
                elif ncopy % 2 == 0:
                    ot = outp.tile([128, TB], BF16, name="ot")
                    nc.scalar.copy(ot[:], pots[0][:])
                    nc.scalar.dma_start(ydst, ot[:])
                else:
                    ot = outp.tile([128, TB], BF16, name="ot")
                    nc.vector.tensor_copy(ot[:], pots[0][:])
                    nc.sync.dma_start(ydst, ot[:])
                ncopy += 1


_NC_CACHE = None


def build_nc():
    global _NC_CACHE
    if _NC_CACHE is not None:
        return _NC_CACHE
    nc = bacc.Bacc(
        "TRN2", target_bir_lowering=False, debug=False, num_devices=NCORES
    )
    xh = nc.dram_tensor("xh", [128, NCHUNK, XC], BF16, kind="ExternalInput").ap()
    xgh = nc.dram_tensor(
        "xgh", [128, NCHUNK * (PAD + TB)], BF16, kind="ExternalInput"
    ).ap()
    wh = nc.dram_tensor("wh", [128, NCHUNK * WQ], BF16, kind="ExternalInput").ap()
    y = nc.dram_tensor("y", [CPC, TOKC], BF16, kind="ExternalOutput").ap()
    with tile.TileContext(nc) as tc:
        _emit(tc, nc, xh, xgh, wh, y)
    nc.compile()
    _NC_CACHE = nc
    return nc


def make_in_maps(x, w):
    """Per-core slices: x channel-major bf16 with leading zero pad per batch;
    w packed as [cin part, (2k+j) block, och] bf16."""
    x = np.ascontiguousarray(x, dtype=np.float32)
    w = np.ascontiguousarray(w, dtype=np.float32)
    in_maps = []
    for core in range(NCORES):
        th, gp = divmod(core, 4)
        cs = slice(gp * CPC, (gp + 1) * CPC)
        # xh[p, ci, b*CB + PAD + tok] = x[2th+b, tok, gp*512 + ci*128 + p]
        xcore = np.zeros((128, NCHUNK, XC), dtype=NPBF16)
        xs = x[2 * th:2 * th + 2, :, cs]                     # [2, L, 512]
        xt = (
            xs.transpose(2, 0, 1)                            # [512, 2, L]
            .reshape(NCHUNK, 128, BPC, L)
            .transpose(1, 0, 2, 3)                           # [p, ci, b, tok]
        )
        xcore.reshape(128, NCHUNK, BPC, CB)[:, :, :, PAD:] = xt.astype(NPBF16)
        # wh[p, cc*WQ + (j*K + k)*128 + oc] = w[k, j*128 + p, gp*512 + cc*128 + oc]
        wc = w[:, :, cs]                                     # [K, 256, 512]
        wcore = np.ascontiguousarray(
            wc.reshape(K, 2, 128, NCHUNK, 128)
            .transpose(2, 3, 1, 0, 4)                        # [p, cc, j, k, oc]
            .reshape(128, NCHUNK * WQ)
        ).astype(NPBF16)
        xgcore = np.ascontiguousarray(
            xcore[:, :, 0:PAD + TB].reshape(128, NCHUNK * (PAD + TB))
        )
        in_maps.append({"xh": xcore, "xgh": xgcore, "wh": wcore})
    return in_maps


def run(x, w, trace=False, **kw):
    nc = build_nc()
    res = run_bass_kernel_spmd(
        nc, make_in_maps(x, w), core_ids=list(range(NCORES)), trace=trace, **kw
    )
    out = np.empty((B, L, D), dtype=np.float32)
    for core in range(NCORES):
        th, gp = divmod(core, 4)
        yc = np.asarray(res.results[core]["y"]).astype(np.float32)  # [CPC, BPC*L]
        out[BPC * th: BPC * (th + 1), :, gp * CPC:(gp + 1) * CPC] = (
            yc.reshape(CPC, BPC, L).transpose(1, 2, 0)
        )
    return out, res


def kernel(x, w):
    out, _ = run(x, w, trace=False)
    return out
